# revision 1
# baseline (speedup 1.0000x reference)
"""Contrastive loss (cosine-sim InfoNCE with positive pairs) on 8 TRN2 NeuronCores.

Math: per row i, with sim = cos-sim matrix and tau = 0.08,
  loss = mean_i [ log( sum_j exp(sim_ij/tau) - exp(sim_ii/tau) ) - sim_{i,p(i)}/tau ]
where p(i) is i's positive partner. (The masked denominator pos+row_sums
telescopes to total - diag.)

Sharding: data-parallel over rows. Each core gets the full embeddings (for the
rhs of the Gram matmul) plus its 1024-row slice and the partner-gathered slice
(host-side index plumbing only). Each core computes its [1024 x 8192] slice of
exp(sim/tau) row sums streaming through PSUM (never materializing the matrix),
plus its per-row diag/pos corrections and log terms, and writes a [128,1]
vector of partial loss sums. Host sums 8*128 partials and divides by B.

Numerics: the Gram matmul runs in fp16 (rhs = normalized embeddings, lhsT = raw
rows; the exp's per-partition scale applies rinv_i/tau). The diagonal exp must
cancel against the same value inside the accumulated row total, so it is
recomputed from the *same* fp16 tensors with a DVE dot product.
"""

import numpy as np

import concourse.bacc as bacc
import concourse.bass_utils as bass_utils
import concourse.mybir as mybir
import concourse.tile as tile

B, D = 8192, 128
N_CORES = 8
ROWS = B // N_CORES            # 1024 rows per core
P = 128                        # partitions
T_FULL = B // P                # 64 row-tiles of the full matrix
T_LOC = ROWS // P              # 8 row-tiles per core
N_CHUNK = 512                  # matmul free dim (one PSUM bank)
GRP = 4                        # psum banks per ACT exp instruction
GRP_W = N_CHUNK * GRP          # 2048 columns per ACT instruction
N_GRPS = B // GRP_W            # 4 groups per row-block
BATCH = 8                      # full-preproc row-tiles per pipeline batch
TAU = 0.08

f32 = mybir.dt.float32
f16 = mybir.dt.float16
AF = mybir.ActivationFunctionType
ALU = mybir.AluOpType

_cache = {}

import os
_TMODE = os.environ.get("K_TMODE", "xbar")  # xbar | copy(timing-only) | pe
_SKIP = os.environ.get("K_SKIP", "")  # timing-only bisection: act | mm | loop
_MB = int(os.environ.get("K_MB", str(T_LOC)))    # row-blocks in main loop
_NB = int(os.environ.get("K_NB", str(N_GRPS)))   # col-groups in main loop
# timing-only preproc bisection: "" | scale | norm | all
_SKIP2 = os.environ.get("K_SKIP2", "")
_TENG = os.environ.get("K_TENG", "sync")  # engine issuing xbar transposes


def _teng(nc):
    return nc.scalar if _TENG == "scalar" else nc.sync


_SENG = os.environ.get("K_SENG", "vector")  # engine for scale+cast ops


def _seng(nc):
    return nc.gpsimd if _SENG == "gpsimd" else nc.vector


def _build():
    nc = bacc.Bacc("TRN2", target_bir_lowering=False, debug=False,
                   num_devices=N_CORES)
    ef = nc.dram_tensor("e_full", [B, D], f32, kind="ExternalInput").ap()
    el = nc.dram_tensor("e_loc", [ROWS, D], f32, kind="ExternalInput").ap()
    ep = nc.dram_tensor("e_par", [ROWS, D], f32, kind="ExternalInput").ap()
    out = nc.dram_tensor("partial", [P, 1], f32, kind="ExternalOutput").ap()

    with tile.TileContext(nc) as tc:
        with (
            tc.tile_pool(name="big", bufs=1) as big,
            tc.tile_pool(name="sq", bufs=2) as sqp,
            tc.tile_pool(name="small", bufs=1) as sm,
            tc.tile_pool(name="psum", bufs=2, space="PSUM") as pp,
            tc.tile_pool(name="scr", bufs=2) as scrp,
        ):
            # ---- persistent SBUF tensors ----
            ef32 = big.tile([P, T_FULL, D], f32)       # full E, natural tiles
            en16 = big.tile([P, T_FULL, D], f16)       # normalized fp16
            ent = big.tile([P, B], f16)                # EN^T  (d-part, row-free)
            eloc32 = sm.tile([P, T_LOC, D], f32)
            epar32 = sm.tile([P, T_LOC, D], f32)
            eloc16 = sm.tile([P, T_LOC, D], f16)       # raw local rows, fp16
            enloc16 = sm.tile([P, T_LOC, D], f16)      # normalized local rows
            lhsT = sm.tile([P, ROWS], f16)             # (raw local rows)^T
            ss = sm.tile([P, T_FULL], f32)             # row norms^2 (full)
            rinv = sm.tile([P, T_FULL], f32)           # 1/||e||   (full)
            ss_loc = sm.tile([P, T_LOC], f32)
            ln_loc = sm.tile([P, T_LOC], f32)
            rinv_loc = sm.tile([P, T_LOC], f32)        # 1/||e||      (local)
            rinv_ls = sm.tile([P, T_LOC], f32)         # 1/(tau*||e||) (local)
            ss_par = sm.tile([P, T_LOC], f32)
            rinv_par = sm.tile([P, T_LOC], f32)
            diag = sm.tile([P, T_LOC], f32)            # raw diag dots (fp16 in)
            posdot = sm.tile([P, T_LOC], f32)          # raw pos dots (fp32)
            acc = sm.tile([P, T_LOC * N_GRPS], f32)    # exp row-sums per group
            nc.vector.memset(acc[:], 0.0)
            rtot = sm.tile([P, T_LOC], f32)
            d2 = sm.tile([P, T_LOC], f32)
            dexp = sm.tile([P, T_LOC], f32)
            denom = sm.tile([P, T_LOC], f32)
            lvec = sm.tile([P, T_LOC], f32)
            posfac = sm.tile([P, T_LOC], f32)
            pos2 = sm.tile([P, T_LOC], f32)
            lossv = sm.tile([P, T_LOC], f32)
            part = sm.tile([P, 1], f32)

            neg_ln_tau = sm.tile([P, 1], f32)
            nc.vector.memset(neg_ln_tau[:], float(-np.log(TAU)))

            # ---- local-rows preprocessing (unblocks lhsT + exp scale early) --
            el_r = el.rearrange("(t p) d -> p t d", p=P)
            ep_r = ep.rearrange("(t p) d -> p t d", p=P)
            half = T_LOC // 2
            nc.sync.dma_start(out=eloc32[:, :half, :], in_=el_r[:, :half, :])
            nc.sync.dma_start(out=eloc32[:, half:, :], in_=el_r[:, half:, :])
            nc.sync.dma_start(out=epar32[:], in_=ep_r)

            # lhsT is the raw local rows: cast + transpose immediately so the
            # first matmuls are unblocked by nothing but the small DMA.
            nc.vector.tensor_copy(eloc16[:], eloc32[:])  # fp32 -> fp16 cast
            for m in range(T_LOC):
                if _TMODE == "copy":
                    nc.sync.dma_start(out=lhsT[:, m * P:(m + 1) * P],
                                      in_=eloc16[:, m, :])
                else:
                    _teng(nc).dma_start_transpose(lhsT[:, m * P:(m + 1) * P],
                                                eloc16[:, m, :])

            sql = sqp.tile([P, BATCH, D], f32, tag="sq")
            nc.vector.tensor_mul(sql[:], eloc32[:], eloc32[:])
            nc.vector.reduce_sum(ss_loc[:], sql[:], axis=mybir.AxisListType.X)
            nc.scalar.activation(ln_loc[:], ss_loc[:], AF.Ln)
            # rinv_loc = exp(-0.5*ln(ss));  rinv_ls = rinv_loc / tau
            nc.scalar.activation(rinv_loc[:], ln_loc[:], AF.Exp, scale=-0.5)
            nc.scalar.activation(rinv_ls[:], ln_loc[:], AF.Exp, scale=-0.5,
                                 bias=neg_ln_tau[:])
            for m in range(T_LOC):
                _seng(nc).tensor_scalar_mul(enloc16[:, m, :], eloc32[:, m, :],
                                            rinv_loc[:, m:m + 1])

            # ---- full-matrix preproc batches interleaved with main phases ---
            # Engine streams execute in order, so the per-batch Ln/Exp must be
            # interleaved with the main exp stream or ACT would stall until
            # the whole preproc chain finished. Batches 2g,2g+1 produce ENT
            # tiles 16g..16g+15 = exactly column group g of the main loop.
            ef_r = ef.rearrange("(t p) d -> p t d", p=P)

            def preproc_batch(b):
                if _SKIP2 == "all":
                    return
                t0, t1 = b * BATCH, (b + 1) * BATCH
                tm = t0 + BATCH // 2
                # two DMAs per batch -> parallel HWDGE queues
                nc.sync.dma_start(out=ef32[:, t0:tm, :], in_=ef_r[:, t0:tm, :])
                nc.sync.dma_start(out=ef32[:, tm:t1, :], in_=ef_r[:, tm:t1, :])
                if _SKIP2 == "norm":
                    return
                sqb = sqp.tile([P, BATCH, D], f32, tag="sq")
                nc.vector.tensor_mul(sqb[:], ef32[:, t0:t1, :], ef32[:, t0:t1, :])
                nc.vector.reduce_sum(ss[:, t0:t1], sqb[:],
                                     axis=mybir.AxisListType.X)
                lnb = sqp.tile([P, BATCH], f32, tag="lnb")
                nc.scalar.activation(lnb[:], ss[:, t0:t1], AF.Ln)
                nc.scalar.activation(rinv[:, t0:t1], lnb[:], AF.Exp, scale=-0.5)
                if _SKIP2 == "scale":
                    return
                for t in range(t0, t1):
                    _seng(nc).tensor_scalar_mul(en16[:, t, :], ef32[:, t, :],
                                                rinv[:, t:t + 1])
                for t in range(t0, t1):
                    if _TMODE == "copy":
                        nc.sync.dma_start(out=ent[:, t * P:(t + 1) * P],
                                          in_=en16[:, t, :])
                    else:
                        _teng(nc).dma_start_transpose(ent[:, t * P:(t + 1) * P],
                                                    en16[:, t, :])

            def main_phase(g):
                for m in range(_MB):
                    lhs_m = lhsT[:, m * P:(m + 1) * P]
                    pt = pp.tile([P, GRP_W], f32, tag="pt")
                    for k in range(GRP):
                        n = g * GRP + k
                        nc.tensor.matmul(
                            pt[:, k * N_CHUNK:(k + 1) * N_CHUNK],
                            lhsT=lhs_m,
                            rhs=ent[:, n * N_CHUNK:(n + 1) * N_CHUNK],
                            start=True, stop=True)
                    scr = scrp.tile([P, GRP_W], f32, tag="scr")
                    nc.scalar.activation(
                        scr[:], pt[:], AF.Exp,
                        scale=rinv_ls[:, m:m + 1],
                        accum_out=acc[:, m * N_GRPS + g:m * N_GRPS + g + 1])

            for b in range(T_FULL // BATCH):
                preproc_batch(b)
                if b % 2 == 1 and (b - 1) // 2 < _NB:
                    main_phase((b - 1) // 2)

            # partner-row norms (epilogue-only -> emitted after the main loop
            # so their ACT instrs sit behind the exp stream, not ahead of it)
            sqr = sqp.tile([P, BATCH, D], f32, tag="sq")
            nc.vector.tensor_mul(sqr[:], epar32[:], epar32[:])
            nc.vector.reduce_sum(ss_par[:], sqr[:], axis=mybir.AxisListType.X)
            lnp = sqp.tile([P, T_LOC], f32, tag="lnp")
            nc.scalar.activation(lnp[:], ss_par[:], AF.Ln)
            nc.scalar.activation(rinv_par[:], lnp[:], AF.Exp, scale=-0.5)

            # raw diag dots over the same fp16 values the matmul sees
            # (tensor_tensor_reduce crashes this runtime; use mul+reduce).
            # Emitted after the main loop so DVE prioritizes ENT production.
            dprod = sqp.tile([P, T_LOC, D], f32, tag="sq")
            nc.vector.tensor_mul(dprod[:], eloc16[:], enloc16[:])
            nc.vector.reduce_sum(diag[:], dprod[:], axis=mybir.AxisListType.X)
            # pos dots in fp32 (no cancellation -> no need to match fp16 path)
            pprod = sqp.tile([P, T_LOC, D], f32, tag="sq")
            nc.vector.tensor_mul(pprod[:], eloc32[:], epar32[:])
            nc.vector.reduce_sum(posdot[:], pprod[:], axis=mybir.AxisListType.X)

            # ---- epilogue: per-row loss, reduce to [128,1] ------------------
            acc_v = acc[:].rearrange("p (m g) -> p m g", g=N_GRPS)
            nc.vector.reduce_sum(rtot[:], acc_v, axis=mybir.AxisListType.X)
            nc.vector.tensor_mul(d2[:], diag[:], rinv_ls[:])
            nc.scalar.activation(dexp[:], d2[:], AF.Exp)
            nc.vector.tensor_tensor(out=denom[:], in0=rtot[:], in1=dexp[:],
                                    op=ALU.subtract)
            nc.scalar.activation(lvec[:], denom[:], AF.Ln)
            nc.vector.tensor_mul(posfac[:], rinv_ls[:], rinv_par[:])
            nc.vector.tensor_mul(pos2[:], posdot[:], posfac[:])
            nc.vector.tensor_tensor(out=lossv[:], in0=lvec[:], in1=pos2[:],
                                    op=ALU.subtract)
            nc.vector.reduce_sum(part[:], lossv[:], axis=mybir.AxisListType.X)
            nc.sync.dma_start(out=out, in_=part[:])

    nc.compile()
    return nc


def _get_nc():
    if "nc" not in _cache:
        _cache["nc"] = _build()
    return _cache["nc"]


def kernel(embeddings, positive_pairs):
    E = np.ascontiguousarray(np.asarray(embeddings), dtype=np.float32)
    pp = np.asarray(positive_pairs)
    assert E.shape == (B, D)

    partner = np.full(B, -1, dtype=np.int64)
    i, j = pp[:, 0].astype(np.int64), pp[:, 1].astype(np.int64)
    partner[i] = j
    partner[j] = i
    assert (partner >= 0).all(), "positive_pairs must cover every row"

    nc = _get_nc()
    in_maps = []
    for c in range(N_CORES):
        rows = np.arange(c * ROWS, (c + 1) * ROWS)
        in_maps.append({
            "e_full": E,
            "e_loc": E[rows],
            "e_par": np.ascontiguousarray(E[partner[rows]]),
        })
    res = bass_utils.run_bass_kernel_spmd(nc, in_maps,
                                          core_ids=list(range(N_CORES)))
    total = sum(float(res.results[c]["partial"].sum()) for c in range(N_CORES))
    return np.float32(total / B)

